# revision 1
# baseline (speedup 1.0000x reference)
"""Trainium2 Bass kernel for Transformer-XL style relative-position MHSA.

Problem: nn_MultiHeadSelfAttention_14989435863450
  B=2, S=2048, D=512, H=8, dh=64, fp32 I/O.

Sharding (8 cores): core c -> batch b = c//4, head pair h0 = 2*(c%4).
Each core computes its 2 heads' attention and the partial output
projection; host sums 4 partials per batch and adds (bv @ Wo + bo).

Math folds (exact):
  - bq folds into u,v:  u_eff = (u + bq) / sqrt(D)
  - bk adds a per-query-row constant to scores -> cancels in softmax
  - bv contributes attn-weighted 1 * bv = bv -> host-side constant
  - 1/sqrt(D) folded into q at evacuation time

Design (transposed attention, kt-outer):
  - All matmul operands bf16 (f32r streams ~2 cyc/row; bf16 at 1).
  - Scores built TRANSPOSED: sT[k, q] = kT.T @ qTu per 128-k tile; the
    attn matrix never needs a PE transpose before the attn@v matmul.
  - Rel-shift via DRAM buffer PB[S, S+1] (rows [0 | posrow_i]); the
    shifted [q, k] view (flat[S + q*S + k]) is read back TRANSPOSED by
    the XBAR DMA in [1024, 128] panels (one per (h, q-half, k-tile)).
    ALL transposes ride the sync HWDGE ring: concurrent XBAR transposes
    on both rings race on the shared transpose unit and corrupt data
    (found via run-to-run output jitter; single-ring is bit-stable).
  - All pos blocks (both heads jointly) run in the prologue, and each
    half's 16 panel reads are issued up front, so no shifted read ever
    waits on a PB write (whole-tile dep tracking would otherwise chain
    reads behind unrelated later writes).
  - kt-outer loop per q-half: po accumulates into a 2-bank PSUM tile
    [65, 2*512]; attn@v for k-tile kt-1 issues one step late so the
    exp(kt-1) latency is off the PE critical path. psC depth 3.
  - Softmax denominator rides as a ones-column in vv: po row 64 is Z.
    Unnormalized o2u evacuates per half; Z bounces through DRAM and an
    XBAR transpose into [128, 16] (q-partitioned), one cheap full-lane
    reciprocal, and the per-head 1/Z folds into the output projection
    as a per-partition scalar (pw_h kept in separate PSUM slots).
"""

import math
from contextlib import ExitStack

import numpy as np
import ml_dtypes

import concourse.bass as bass
import concourse.bacc as bacc_mod
import concourse.mybir as mybir
import concourse.tile as tile
from concourse.bass import ts, ds
from concourse.bass_utils import run_bass_kernel_spmd

FP32 = mybir.dt.float32
F32R = mybir.dt.float32r
BF16 = mybir.dt.bfloat16
FP8 = mybir.dt.float8e4
DR = mybir.MatmulPerfMode.DoubleRow

D_MODEL = 512
NUM_HEADS = 8
D_HEAD = 64
DH2 = 2 * D_HEAD
B_FULL = 2
S_FULL = 2048
P = 128
CH = 512
ISQ = 1.0 / math.sqrt(D_MODEL)

Exp = mybir.ActivationFunctionType.Exp
ADD = mybir.AluOpType.add
MULT = mybir.AluOpType.mult


def build_nc(S=S_FULL):
    nc = bacc_mod.Bacc()
    NB = S // P        # 16 q blocks
    NK = S // P        # 16 k tiles
    NCH = S // CH      # 4 chunks
    KD = D_MODEL // P  # 4
    HALF = S // 2      # 1024

    xT = nc.declare_dram_parameter("xT", [D_MODEL, S], BF16, isOutput=False)
    posT = nc.declare_dram_parameter("posT", [D_MODEL, S], BF16, isOutput=False)
    Wq = nc.declare_dram_parameter("Wq", [D_MODEL, DH2], BF16, isOutput=False)
    Wk = nc.declare_dram_parameter("Wk", [D_MODEL, DH2], BF16, isOutput=False)
    Wv = nc.declare_dram_parameter("Wv", [D_MODEL, DH2], BF16, isOutput=False)
    Wp = nc.declare_dram_parameter("Wp", [D_MODEL, DH2], BF16, isOutput=False)
    Wo = nc.declare_dram_parameter("Wo", [DH2, D_MODEL], BF16, isOutput=False)
    ueff = nc.declare_dram_parameter("ueff", [DH2, 1], FP32, isOutput=False)
    veff = nc.declare_dram_parameter("veff", [DH2, 1], FP32, isOutput=False)
    out_partial = nc.declare_dram_parameter("out_partial", [S, D_MODEL], FP32, isOutput=True)

    with ExitStack() as ctx:
        tc = ctx.enter_context(tile.TileContext(nc))
        consts = ctx.enter_context(tc.tile_pool(name="consts", bufs=1))
        blk = ctx.enter_context(tc.tile_pool(name="blk", bufs=3))
        spool = ctx.enter_context(tc.tile_pool(name="spool", bufs=18))
        dram = ctx.enter_context(tc.tile_pool(name="dram", bufs=1, space="DRAM"))
        # PSUM (8 banks): psAcc 1x[65,2,512] (2) + psC 3x[128,2,512] (6)
        psAcc = ctx.enter_context(tc.tile_pool(name="psAcc", bufs=1, space="PSUM"))
        psC = ctx.enter_context(tc.tile_pool(name="psC", bufs=3, space="PSUM"))

        # ---- load constants / inputs ----
        xT_sb = consts.tile([P, KD, S], BF16)
        nc.sync.dma_start(xT_sb[:], xT.rearrange("(o p) s -> p o s", p=P))
        posT_sb = consts.tile([P, KD, S], BF16)
        nc.sync.dma_start(posT_sb[:], posT.rearrange("(o p) s -> p o s", p=P))
        w_sbs = {}
        for nm, handle in (("Wq", Wq), ("Wp", Wp), ("Wk", Wk), ("Wv", Wv)):
            w_sb = consts.tile([P, KD, DH2], BF16, name=f"{nm}_sb")
            nc.sync.dma_start(w_sb[:], handle.rearrange("(o p) m -> p o m", p=P))
            w_sbs[nm] = w_sb
        Wo_sb = consts.tile([D_HEAD, 2, D_MODEL], BF16)
        nc.sync.dma_start(Wo_sb[:], Wo.rearrange("(h d) n -> d h n", h=2))
        ueff_sb = consts.tile([DH2, 1], FP32)
        nc.sync.dma_start(ueff_sb[:], ueff[:, :])
        veff_sb = consts.tile([DH2, 1], FP32)
        nc.sync.dma_start(veff_sb[:], veff[:, :])

        qTu = consts.tile([DH2, S], BF16)
        qTv = consts.tile([DH2, S], BF16)
        kT = consts.tile([DH2, S], BF16)
        pT = consts.tile([DH2, S], BF16)
        vv_aug = consts.tile([P, NK, 2, D_HEAD + 1], BF16)
        ones_st = consts.tile([P, NK * 2], FP32)
        nc.vector.memset(ones_st[:], 1.0)
        nc.vector.tensor_copy(
            vv_aug[:, :, :, D_HEAD : D_HEAD + 1],
            ones_st[:].rearrange("p (a b c) -> p a b c", a=NK, b=2),
        )

        def proj_groups(w_sb, src_sb, evac):
            for g in range(NCH // 2):
                pg = psC.tile([P, 2, CH], FP32, tag="ps", name="pg")
                for j in range(2):
                    chn = 2 * g + j
                    for kt in range(KD):
                        nc.tensor.matmul(
                            pg[:, j, :],
                            lhsT=w_sb[:, kt, :],
                            rhs=src_sb[:, kt, ts(chn, CH)],
                            start=(kt == 0),
                            stop=(kt == KD - 1),
                        )
                evac(g, pg)

        def evac_q(g, pg):
            sl = ds(g * 2 * CH, 2 * CH)
            pv = pg[:].rearrange("p a b -> p (a b)")
            nc.vector.tensor_scalar(qTu[:, sl], pv, ISQ, ueff_sb[:, 0:1], MULT, ADD)
            nc.vector.tensor_scalar(qTv[:, sl], pv, ISQ, veff_sb[:, 0:1], MULT, ADD)

        def evac_to(dst):
            def evac(g, pg):
                sl = ds(g * 2 * CH, 2 * CH)
                nc.scalar.copy(dst[:, sl], pg[:].rearrange("p a b -> p (a b)"))
            return evac

        def proj_v():
            for sg in range(NK // 2):
                pv = psC.tile([P, 2, CH], FP32, tag="ps", name="pv")
                for j in range(2):
                    st = 2 * sg + j
                    for kt in range(KD):
                        nc.tensor.matmul(
                            pv[:, j, 0:DH2],
                            lhsT=xT_sb[:, kt, ts(st, P)],
                            rhs=w_sbs["Wv"][:, kt, :],
                            start=(kt == 0),
                            stop=(kt == KD - 1),
                        )
                for j in range(2):
                    st = 2 * sg + j
                    src = pv[:, j, 0:DH2].rearrange("p (h d) -> p h d", h=2)
                    nc.vector.tensor_copy(vv_aug[:, st, :, 0:D_HEAD], src)

        PB = [dram.tile([S, S + 1], BF16, name=f"pb{h}") for h in range(2)]

        def pos_block(ib):
            """pos scores (orientation A) for q rows [128*ib, +128), BOTH
            heads jointly: the two heads' matmuls sit at PE array rows 0-63
            and 64-127 (tile_position), so consecutive pairs overlap."""
            pes = [
                blk.tile([P, S + 1], BF16, tag=f"posext{h}", name="pe")
                for h in range(2)
            ]
            for h in range(2):
                nc.vector.memset(pes[h][:, 0:1], 0.0)
            for g in range(NCH // 2):
                pps = [
                    psC.tile([P, 2, CH], FP32, tag="ps", name="pp")
                    for _ in range(2)
                ]
                for h in range(2):
                    for j in range(2):
                        chn = 2 * g + j
                        nc.tensor.matmul(
                            pps[h][:, j, :],
                            lhsT=qTv[ds(h * D_HEAD, D_HEAD), ts(ib, P)],
                            rhs=pT[ds(h * D_HEAD, D_HEAD), ts(chn, CH)],
                            start=True,
                            stop=True,
                        )
                for h in range(2):
                    dst = pes[h][:, ds(1 + g * 2 * CH, 2 * CH)]
                    src = pps[h][:].rearrange("p a b -> p (a b)")
                    if (h + g) % 2 == 0:
                        nc.scalar.copy(dst, src)
                    else:
                        nc.vector.tensor_copy(dst, src)
            for h in range(2):
                (nc.sync if h == 0 else nc.scalar).dma_start(
                    PB[h][ts(ib, P), :], pes[h][:]
                )

        # unnormalized attn@v results (row 64 = softmax denominator Z)
        o2u = {}
        rz = {}
        for h in range(2):
            o2u[h] = blk.tile([D_HEAD + 1, NCH, CH], BF16, tag=f"o2_{h}", name="o2u")
            rz[h] = blk.tile([P, NB], FP32, tag=f"rz_{h}", name="rz")
        zd = dram.tile([2, S], BF16, name="zd")

        def issue_read(h, half, kt):
            """prefetch the shifted+transposed pos panel for (h, half, kt)."""
            sp = spool.tile([P, 2, CH], BF16, tag="spos", name="sp")
            flat = PB[h].flatten()
            qview = flat[ds(S + half * HALF * S, HALF * S)].rearrange(
                "(q k) -> q k", k=S
            )
            nc.sync.dma_start(sp[:].rearrange("p a b -> p (a b)"),
                              qview[:, ts(kt, P)], transpose=True)
            return sp

        def po_step(h, kt, po, et):
            for j in range(2):
                nc.tensor.matmul(
                    po[:, j, :],
                    lhsT=vv_aug[:, kt, h, :],
                    rhs=et[:, j, :],
                    start=(kt == 0),
                    stop=(kt == NK - 1),
                )

        def kt_step(h, half, kt, po, sp, prev, interleave):
            """content scores + exp for k-tile kt; attn@v for k-tile kt-1
            (delayed one step so exp(kt-1) is off the PE critical path)."""
            ps = psC.tile([P, 2, CH], FP32, tag="ps", name="ps")
            for j in range(2):
                c = 2 * half + j
                nc.tensor.matmul(
                    ps[:, j, :],
                    lhsT=kT[ds(h * D_HEAD, D_HEAD), ts(kt, P)],
                    rhs=qTu[ds(h * D_HEAD, D_HEAD), ts(c, CH)],
                    start=True,
                    stop=True,
                )
            if interleave:
                interleave.pop(0)()
            if prev is not None:
                po_step(h, prev[0], po, prev[1])
            sc = blk.tile([P, 2, CH], BF16, tag="sc", name="sc")
            nc.vector.tensor_tensor(sc[:], ps[:], sp[:], ADD)
            et = blk.tile([P, 2, CH], BF16, tag="et", name="et")
            nc.scalar.activation(et[:], sc[:], Exp)
            return (kt, et)

        def evac_half(h, half, po):
            """po [65, 2, 512] -> o2u[h] chunks of this half (incl. Z row)."""
            dst = o2u[h][:, ts(half, 2), :]
            if (h + half) % 2 == 0:
                nc.vector.tensor_copy(dst, po[:])
            else:
                nc.scalar.copy(dst, po[:])

        def finish_head(h):
            """Z row -> DRAM -> xbar-transposed [128, 16] -> rz = 1/Z."""
            nc.scalar.dma_start(
                zd[h : h + 1, :],
                o2u[h][D_HEAD : D_HEAD + 1, :, :].rearrange("p a b -> p (a b)"),
            )
            zview = zd.flatten()[ds(h * S, S)].rearrange("(a b) -> a b", b=P)
            rzt = blk.tile([P, NB], BF16, tag=f"rzt_{h}", name="rzt")
            nc.sync.dma_start(rzt[:], zview, transpose=True)
            nc.vector.reciprocal(rz[h][:], rzt[:])

        def pw_block(ib):
            c, j = ib // NCH, ib % NCH
            pw = psC.tile([P, 2, CH], FP32, tag="ps", name="pw")
            for h in range(2):
                nc.tensor.matmul(
                    pw[:, h, :],
                    lhsT=o2u[h][0:D_HEAD, c, ts(j, P)],
                    rhs=Wo_sb[:, h, :],
                    start=True,
                    stop=True,
                )
            t1 = blk.tile([P, D_MODEL], FP32, tag="t1", name="t1")
            nc.scalar.mul(t1[:], pw[:, 1, :], rz[1][:, ib : ib + 1])
            fin = blk.tile([P, D_MODEL], FP32, tag="fin", name="fin")
            nc.vector.scalar_tensor_tensor(
                fin[:], pw[:, 0, :], rz[0][:, ib : ib + 1], t1[:], MULT, ADD
            )
            nc.scalar.dma_start(out_partial[ts(ib, P), :], fin[:])

        # ---- prologue: projections + ALL pos blocks (so the main loop's
        # shifted reads never wait on a PB write) ----
        proj_groups(w_sbs["Wq"], xT_sb, evac_q)
        proj_groups(w_sbs["Wp"], posT_sb, evac_to(pT))
        pre = [
            (lambda: proj_groups(w_sbs["Wk"], xT_sb, evac_to(kT))),
            (lambda: proj_v()),
        ]
        for ib in range(NB):
            pos_block(ib)
            if pre:
                pre.pop(0)()

        # ---- main: per head, kt-outer within each q-half ----
        for h in range(2):
            for half in range(2):
                po = psAcc.tile([D_HEAD + 1, 2, CH], FP32, tag="po", name="po")
                # issue ALL reads first: later pos-block writes then cannot
                # create whole-tile false deps on this half's reads
                sps = {kt: issue_read(h, half, kt) for kt in range(NK)}
                inter = []
                prev = None
                for kt in range(NK):
                    prev = kt_step(h, half, kt, po, sps.pop(kt), prev, inter)
                po_step(h, prev[0], po, prev[1])
                for fn in inter:
                    fn()
                evac_half(h, half, po)
            finish_head(h)

        for ib in range(NB):
            pw_block(ib)

    nc.finalize()
    return nc


# ---------------- host side ----------------

_NC_CACHE = {}


def _get_nc(S=S_FULL):
    if S not in _NC_CACHE:
        _NC_CACHE[S] = build_nc(S)
    return _NC_CACHE[S]


def make_in_maps(inputs, S=S_FULL, n_cores=8):
    bf16 = ml_dtypes.bfloat16
    x = np.asarray(inputs["x"], np.float32)
    pos = np.asarray(inputs["pos_embedding"], np.float32)
    Wq = np.asarray(inputs["Wq"], np.float32)
    bq = np.asarray(inputs["bq"], np.float32)
    Wk = np.asarray(inputs["Wk"], np.float32)
    Wv = np.asarray(inputs["Wv"], np.float32)
    Wp = np.asarray(inputs["Wp"], np.float32)
    u = np.asarray(inputs["u"], np.float32)
    v = np.asarray(inputs["v"], np.float32)
    Wo = np.asarray(inputs["Wo"], np.float32)

    xTb = [np.ascontiguousarray(x[b, :S].T).astype(bf16) for b in range(B_FULL)]
    posTb = [np.ascontiguousarray(pos[b, :S].T).astype(bf16) for b in range(B_FULL)]

    in_maps = []
    for c in range(n_cores):
        b = c // 4
        h0 = 2 * (c % 4)
        sl = slice(h0 * D_HEAD, (h0 + 2) * D_HEAD)
        u_eff = ((u[h0 : h0 + 2].reshape(-1) + bq[sl]) * ISQ).astype(np.float32)
        v_eff = ((v[h0 : h0 + 2].reshape(-1) + bq[sl]) * ISQ).astype(np.float32)
        in_maps.append(
            {
                "xT": xTb[b],
                "posT": posTb[b],
                "Wq": np.ascontiguousarray(Wq[:, sl]).astype(bf16),
                "Wk": np.ascontiguousarray(Wk[:, sl]).astype(bf16),
                "Wv": np.ascontiguousarray(Wv[:, sl]).astype(bf16),
                "Wp": np.ascontiguousarray(Wp[:, sl]).astype(bf16),
                "Wo": np.ascontiguousarray(Wo[sl, :]).astype(bf16),
                "ueff": u_eff.reshape(DH2, 1),
                "veff": v_eff.reshape(DH2, 1),
            }
        )
    return in_maps


def assemble(inputs, results, S=S_FULL):
    bv = np.asarray(inputs["bv"], np.float64)
    Wo = np.asarray(inputs["Wo"], np.float64)
    bo = np.asarray(inputs["bo"], np.float64)
    const = (bv @ Wo + bo).astype(np.float32)
    out = np.zeros((B_FULL, S, D_MODEL), np.float32)
    for c, res in enumerate(results):
        out[c // 4] += res["out_partial"]
    out += const[None, None, :]
    return out


def _run(inputs, trace=False, **kw):
    nc = _get_nc(S_FULL)
    in_maps = make_in_maps(inputs, S_FULL)
    res = run_bass_kernel_spmd(nc, in_maps, list(range(8)), trace=trace, **kw)
    out = assemble(inputs, res.results, S_FULL)
    return out, res


def kernel(**inputs) -> np.ndarray:
    out, _ = _run(inputs, trace=False)
    return out



# revision 4
# speedup vs baseline: 1.1670x; 1.1670x over previous
"""Trainium2 Bass kernel for Transformer-XL style relative-position MHSA.

Problem: nn_MultiHeadSelfAttention_14989435863450
  B=2, S=2048, D=512, H=8, dh=64, fp32 I/O.

Sharding (8 cores): core c -> batch b = c//4, head pair h0 = 2*(c%4).
Each core computes its 2 heads' attention and the partial output
projection; host sums 4 partials per batch and adds (bv @ Wo + bo).

Math folds (exact):
  - bq folds into u,v:  u_eff = (u + bq) / sqrt(D)
  - bk adds a per-query-row constant to scores -> cancels in softmax
  - bv contributes attn-weighted 1 * bv = bv -> host-side constant
  - 1/sqrt(D) folded into q at evacuation time

v2 design (pipelined waves; see kernel_baseline.py for the v1 layout):
  - Scores built TRANSPOSED (sT[k, q]) so attn@v needs no transpose.
  - Rel-shift via DRAM bounce, but PB is SPLIT at the half boundary:
    PBa[h] rows 0..1151 (q-blocks 0-8) serves half 0's shifted reads,
    PBb[h] rows 1024..2047 (blocks 8-15, block 8 written twice) serves
    half 1.  Reads of one half therefore never chain behind writes for
    the other half (whole-tile dep tracking).
  - Ring assignment: sync HWDGE ring carries ONLY the XBAR transposed
    panel reads (concurrent XBAR transposes on both rings race on the
    shared transpose unit -> single ring).  h0's PB writes ride the
    gpsimd SWDGE queue, h1's the scalar HWDGE ring; input loads + out
    writes also scalar.  This kills the v1 serialization where the
    whole main loop queued behind all 32 PB writes on one ring.
  - 4 waves (h0,f0) (h1,f0) (h0,f1) (h1,f1), each: 16 XBAR panel reads
    issued up front, then 16 kt-steps (content matmul -> +pos panel add
    (DVE, every 4th on gpsimd) -> exp (Act) -> attn@v one step late so
    exp latency is off the PE critical path).  Wave k+1's pos blocks /
    projections are interleaved into earlier waves' kt-steps to keep
    the PE continuously busy (pstate ramps to 2.4GHz after ~3us busy;
    idle drops it to 1.2GHz - the v1 killer).
  - Input x/pos loaded as 4 per-chunk SBUF tiles so projections start
    while later chunks stream.
  - out_partial is bf16 (host accumulates in fp32): halves out traffic.
"""

import math
from contextlib import ExitStack

import numpy as np
import ml_dtypes

import concourse.bass as bass
import concourse.bacc as bacc_mod
import concourse.mybir as mybir
import concourse.tile as tile
from concourse.bass import ts, ds
from concourse.bass_utils import run_bass_kernel_spmd

FP32 = mybir.dt.float32
BF16 = mybir.dt.bfloat16

D_MODEL = 512
NUM_HEADS = 8
D_HEAD = 64
DH2 = 2 * D_HEAD
B_FULL = 2
S_FULL = 2048
P = 128
CH = 512
ISQ = 1.0 / math.sqrt(D_MODEL)

Exp = mybir.ActivationFunctionType.Exp
ADD = mybir.AluOpType.add
MULT = mybir.AluOpType.mult


def build_nc(S=S_FULL):
    nc = bacc_mod.Bacc()
    NB = S // P        # 16 q blocks
    NK = S // P        # 16 k tiles
    NCH = S // CH      # 4 chunks
    KD = D_MODEL // P  # 4
    HALF = S // 2      # 1024
    NBH = NB // 2      # 8 q blocks per half

    xT = nc.declare_dram_parameter("xT", [D_MODEL, S], BF16, isOutput=False)
    posT = nc.declare_dram_parameter("posT", [D_MODEL, S], BF16, isOutput=False)
    Wq = nc.declare_dram_parameter("Wq", [D_MODEL, DH2], BF16, isOutput=False)
    Wk = nc.declare_dram_parameter("Wk", [D_MODEL, DH2], BF16, isOutput=False)
    Wv = nc.declare_dram_parameter("Wv", [D_MODEL, DH2], BF16, isOutput=False)
    Wp = nc.declare_dram_parameter("Wp", [D_MODEL, DH2], BF16, isOutput=False)
    Wo = nc.declare_dram_parameter("Wo", [DH2, D_MODEL], BF16, isOutput=False)
    ueff = nc.declare_dram_parameter("ueff", [DH2, 1], FP32, isOutput=False)
    veff = nc.declare_dram_parameter("veff", [DH2, 1], FP32, isOutput=False)
    out_partial = nc.declare_dram_parameter("out_partial", [S, D_MODEL], BF16, isOutput=True)

    with ExitStack() as ctx:
        tc = ctx.enter_context(tile.TileContext(nc))
        consts = ctx.enter_context(tc.tile_pool(name="consts", bufs=1))
        blk = ctx.enter_context(tc.tile_pool(name="blk", bufs=3))
        spool = ctx.enter_context(tc.tile_pool(name="spool", bufs=18))
        dram = ctx.enter_context(tc.tile_pool(name="dram", bufs=1, space="DRAM"))
        # PSUM (8 banks): psAcc 1x[65,2,512] (2) + psC 3x[128,2,512] (6)
        psAcc = ctx.enter_context(tc.tile_pool(name="psAcc", bufs=1, space="PSUM"))
        psC = ctx.enter_context(tc.tile_pool(name="psC", bufs=3, space="PSUM"))

        # ---- loads: all on the scalar HWDGE ring, chunked for early start
        w_sbs = {}

        def load_w(nm, handle):
            w_sb = consts.tile([P, KD, DH2], BF16, name=f"{nm}_sb")
            nc.scalar.dma_start(w_sb[:], handle.rearrange("(o p) m -> p o m", p=P))
            w_sbs[nm] = w_sb

        xc = []
        pc = []
        load_w("Wq", Wq)
        xv = xT.rearrange("(o p) s -> p o s", p=P)
        pv = posT.rearrange("(o p) s -> p o s", p=P)
        for c in range(NCH):
            t = consts.tile([P, KD, CH], BF16, name=f"xc{c}")
            nc.scalar.dma_start(t[:], xv[:, :, ts(c, CH)])
            xc.append(t)
        load_w("Wp", Wp)
        for c in range(NCH):
            t = consts.tile([P, KD, CH], BF16, name=f"pc{c}")
            nc.scalar.dma_start(t[:], pv[:, :, ts(c, CH)])
            pc.append(t)
        load_w("Wk", Wk)
        load_w("Wv", Wv)
        Wo_sb = consts.tile([D_HEAD, 2, D_MODEL], BF16)
        nc.scalar.dma_start(Wo_sb[:], Wo.rearrange("(h d) n -> d h n", h=2))
        ueff_sb = consts.tile([DH2, 1], FP32)
        nc.scalar.dma_start(ueff_sb[:], ueff[:, :])
        veff_sb = consts.tile([DH2, 1], FP32)
        nc.scalar.dma_start(veff_sb[:], veff[:, :])

        qTu = consts.tile([DH2, S], BF16)
        qTv = consts.tile([DH2, S], BF16)
        kT = consts.tile([DH2, S], BF16)
        pT = consts.tile([DH2, S], BF16)
        vv_aug = consts.tile([P, NK, 2, D_HEAD + 1], BF16)
        ones_st = consts.tile([P, NK * 2], FP32)
        nc.vector.memset(ones_st[:], 1.0)
        nc.vector.tensor_copy(
            vv_aug[:, :, :, D_HEAD : D_HEAD + 1],
            ones_st[:].rearrange("p (a b c) -> p a b c", a=NK, b=2),
        )

        # ---- projections (per group g of 2 chunks) ----
        def proj_group(w_sb, src, g, evac):
            pg = psC.tile([P, 2, CH], FP32, tag="ps", name="pg")
            for j in range(2):
                chn = 2 * g + j
                for kt in range(KD):
                    nc.tensor.matmul(
                        pg[:, j, :],
                        lhsT=w_sb[:, kt, :],
                        rhs=src[chn][:, kt, :],
                        start=(kt == 0),
                        stop=(kt == KD - 1),
                    )
            evac(g, pg)

        def evac_q(g, pg):
            sl = ds(g * 2 * CH, 2 * CH)
            pv_ = pg[:].rearrange("p a b -> p (a b)")
            nc.vector.tensor_scalar(qTu[:, sl], pv_, ISQ, ueff_sb[:, 0:1], MULT, ADD)
            nc.vector.tensor_scalar(qTv[:, sl], pv_, ISQ, veff_sb[:, 0:1], MULT, ADD)

        def evac_to(dst):
            def evac(g, pg):
                sl = ds(g * 2 * CH, 2 * CH)
                nc.scalar.copy(dst[:, sl], pg[:].rearrange("p a b -> p (a b)"))
            return evac

        def proj_v(sg):
            pvv = psC.tile([P, 2, CH], FP32, tag="ps", name="pvv")
            for j in range(2):
                st = 2 * sg + j
                for kt in range(KD):
                    nc.tensor.matmul(
                        pvv[:, j, 0:DH2],
                        lhsT=xc[st // 4][:, kt, ts(st % 4, P)],
                        rhs=w_sbs["Wv"][:, kt, :],
                        start=(kt == 0),
                        stop=(kt == KD - 1),
                    )
            for j in range(2):
                st = 2 * sg + j
                src = pvv[:, j, 0:DH2].rearrange("p (h d) -> p h d", h=2)
                nc.vector.tensor_copy(vv_aug[:, st, :, 0:D_HEAD], src)

        # ---- pos score DRAM buffers, split at the half boundary ----
        # PBa[h]: q rows 0..(NBH+1)*P-1  (blocks 0..8)  -> half-0 reads
        # PBb[h]: q rows HALF..S-1       (blocks 8..15) -> half-1 reads
        PBa = [dram.tile([(NBH + 1) * P, S + 1], BF16, name=f"pba{h}") for h in range(2)]
        PBb = [dram.tile([HALF, S + 1], BF16, name=f"pbb{h}") for h in range(2)]

        def pos_block(ib, evac_engines=("vector", "scalar")):
            """pos scores for q rows [128*ib, +128), BOTH heads jointly:
            the two heads' matmuls sit at PE array rows 0-63 and 64-127
            (lhsT partition offset), so consecutive pairs overlap."""
            pes = [
                blk.tile([P, S + 1], BF16, tag=f"posext{h}", name="pe")
                for h in range(2)
            ]
            for h in range(2):
                nc.vector.memset(pes[h][:, 0:1], 0.0)
            for g in range(NCH // 2):
                pps = [
                    psC.tile([P, 2, CH], FP32, tag="ps", name="pp")
                    for _ in range(2)
                ]
                for h in range(2):
                    for j in range(2):
                        chn = 2 * g + j
                        nc.tensor.matmul(
                            pps[h][:, j, :],
                            lhsT=qTv[ds(h * D_HEAD, D_HEAD), ts(ib, P)],
                            rhs=pT[ds(h * D_HEAD, D_HEAD), ts(chn, CH)],
                            start=True,
                            stop=True,
                        )
                for h in range(2):
                    dst = pes[h][:, ds(1 + g * 2 * CH, 2 * CH)]
                    src = pps[h][:].rearrange("p a b -> p (a b)")
                    eng = evac_engines[(h + g) % 2]
                    getattr(nc, eng).tensor_copy(dst, src) if eng in (
                        "vector",
                        "gpsimd",
                    ) else nc.scalar.copy(dst, src)
            # writes: h0 -> gpsimd SWDGE, h1 -> scalar HWDGE ring
            if ib <= NBH:
                nc.gpsimd.dma_start(PBa[0][ts(ib, P), :], pes[0][:])
                nc.scalar.dma_start(PBa[1][ts(ib, P), :], pes[1][:])
            if ib >= NBH:
                nc.gpsimd.dma_start(PBb[0][ts(ib - NBH, P), :], pes[0][:])
                nc.scalar.dma_start(PBb[1][ts(ib - NBH, P), :], pes[1][:])

        # unnormalized attn@v results per head (d rows), Z staged separately
        o2u = {}
        zq = {}
        rz = {}
        for h in range(2):
            o2u[h] = blk.tile([D_HEAD, NCH, CH], BF16, tag=f"o2_{h}", bufs=1, name="o2u")
            zq[h] = blk.tile([1, NCH, CH], BF16, tag=f"zq_{h}", bufs=1, name="zq")
            rz[h] = blk.tile([P, NB], FP32, tag=f"rz_{h}", bufs=1, name="rz")
        zd = dram.tile([2, S], BF16, name="zd")

        def issue_read(h, half, kt):
            """prefetch the shifted+transposed pos panel for (h, half, kt)."""
            sp = spool.tile([P, 2, CH], BF16, tag="spos", name="sp")
            if half == 0:
                flat = PBa[h].flatten()
                qview = flat[ds(S, HALF * S)].rearrange("(q k) -> q k", k=S)
            else:
                flat = PBb[h].flatten()
                qview = flat[ds(HALF, HALF * S)].rearrange("(q k) -> q k", k=S)
            nc.sync.dma_start(sp[:].rearrange("p a b -> p (a b)"),
                              qview[:, ts(kt, P)], transpose=True)
            return sp

        def po_step(h, kt, po, et):
            for j in range(2):
                nc.tensor.matmul(
                    po[:, j, :],
                    lhsT=vv_aug[:, kt, h, :],
                    rhs=et[:, j, :],
                    start=(kt == 0),
                    stop=(kt == NK - 1),
                )

        def kt_step(h, half, kt, po, sp, prev, inter):
            """content scores + exp for k-tile kt; attn@v for k-tile kt-1
            (delayed one step so exp(kt-1) is off the PE critical path)."""
            ps = psC.tile([P, 2, CH], FP32, tag="ps", name="ps")
            for j in range(2):
                c = 2 * half + j
                nc.tensor.matmul(
                    ps[:, j, :],
                    lhsT=kT[ds(h * D_HEAD, D_HEAD), ts(kt, P)],
                    rhs=qTu[ds(h * D_HEAD, D_HEAD), ts(c, CH)],
                    start=True,
                    stop=True,
                )
            for fn in inter:
                fn()
            if prev is not None:
                po_step(h, prev[0], po, prev[1])
            sc = blk.tile([P, 2, CH], BF16, tag="sc", name="sc")
            nc.vector.tensor_tensor(sc[:], ps[:], sp[:], ADD)
            et = blk.tile([P, 2, CH], BF16, tag="et", name="et")
            nc.scalar.activation(et[:], sc[:], Exp)
            return (kt, et)

        def wave(h, half, inter_map):
            po = psAcc.tile([D_HEAD + 1, 2, CH], FP32, tag="po", name="po")
            sps = {kt: issue_read(h, half, kt) for kt in range(NK)}
            prev = None
            for kt in range(NK):
                prev = kt_step(h, half, kt, po, sps.pop(kt), prev,
                               inter_map.get(kt, ()))
            po_step(h, prev[0], po, prev[1])
            # evac: d rows -> o2u (DVE), Z row -> zq (Act)
            nc.vector.tensor_copy(o2u[h][:, ts(half, 2), :], po[0:D_HEAD])
            nc.scalar.copy(zq[h][:, ts(half, 2), :], po[D_HEAD : D_HEAD + 1])

        def finish_head(h):
            """Z -> DRAM -> xbar-transposed [128, 16] -> rz = 1/Z."""
            nc.scalar.dma_start(
                zd[h : h + 1, :], zq[h][:].rearrange("p a b -> p (a b)")
            )
            zview = zd.flatten()[ds(h * S, S)].rearrange("(a b) -> a b", b=P)
            rzt = blk.tile([P, NB], BF16, tag=f"rzt_{h}", bufs=1, name="rzt")
            nc.sync.dma_start(rzt[:], zview, transpose=True)
            nc.vector.reciprocal(rz[h][:], rzt[:])

        def pw_block(ib):
            c, j = ib // NCH, ib % NCH
            pw = psC.tile([P, 2, CH], FP32, tag="ps", name="pw")
            for h in range(2):
                nc.tensor.matmul(
                    pw[:, h, :],
                    lhsT=o2u[h][:, c, ts(j, P)],
                    rhs=Wo_sb[:, h, :],
                    start=True,
                    stop=True,
                )
            t1 = blk.tile([P, D_MODEL], FP32, tag="t1", name="t1")
            nc.scalar.mul(t1[:], pw[:, 1, :], rz[1][:, ib : ib + 1])
            fin = blk.tile([P, D_MODEL], BF16, tag="fin", name="fin")
            nc.vector.scalar_tensor_tensor(
                fin[:], pw[:, 0, :], rz[0][:, ib : ib + 1], t1[:], MULT, ADD
            )
            nc.scalar.dma_start(out_partial[ts(ib, P), :], fin[:])

        # ---- prologue: minimal PE work before pos blocks 0-8 so wave 1
        # (gated on PBa writes) starts as early as possible ----
        proj_group(w_sbs["Wq"], xc, 0, evac_q)
        proj_group(w_sbs["Wp"], pc, 0, evac_to(pT))
        proj_group(w_sbs["Wp"], pc, 1, evac_to(pT))
        pos_block(0)
        proj_group(w_sbs["Wq"], xc, 1, evac_q)   # qTv g1 needed by block 8+
        pos_block(1)
        proj_group(w_sbs["Wk"], xc, 0, evac_to(kT))  # kt 0-7 content
        for ib in range(2, NBH + 1):
            pos_block(ib)

        # ---- waves ----
        w1_inter = {
            0: (lambda: proj_v(0), lambda: proj_v(1)),
            2: (lambda: proj_v(2),),
            4: (lambda: proj_v(3),
                lambda: proj_group(w_sbs["Wk"], xc, 1, evac_to(kT))),
            6: (lambda: proj_v(4),),
            8: (lambda: proj_v(5),),
            10: (lambda: proj_v(6),),
            12: (lambda: proj_v(7),),
        }
        # pos blocks 9-15 spread over wave 2 (writes must land before
        # wave 3's reads of PBb[0]); in-wave evacs go DVE/gpsimd
        w2_inter = {
            1: (lambda: pos_block(9),),
            3: (lambda: pos_block(10),),
            5: (lambda: pos_block(11),),
            7: (lambda: pos_block(12),),
            9: (lambda: pos_block(13),),
            11: (lambda: pos_block(14),),
            13: (lambda: pos_block(15),),
        }
        w4_inter = {
            1: (lambda: finish_head(0),),
        }
        wave(0, 0, w1_inter)
        wave(1, 0, w2_inter)
        wave(0, 1, {})
        wave(1, 1, w4_inter)

        finish_head(1)
        for ib in range(NB):
            pw_block(ib)

    nc.finalize()
    return nc


# ---------------- host side ----------------

_NC_CACHE = {}


def _get_nc(S=S_FULL):
    if S not in _NC_CACHE:
        _NC_CACHE[S] = build_nc(S)
    return _NC_CACHE[S]


def make_in_maps(inputs, S=S_FULL, n_cores=8):
    bf16 = ml_dtypes.bfloat16
    x = np.asarray(inputs["x"], np.float32)
    pos = np.asarray(inputs["pos_embedding"], np.float32)
    Wq = np.asarray(inputs["Wq"], np.float32)
    bq = np.asarray(inputs["bq"], np.float32)
    Wk = np.asarray(inputs["Wk"], np.float32)
    Wv = np.asarray(inputs["Wv"], np.float32)
    Wp = np.asarray(inputs["Wp"], np.float32)
    u = np.asarray(inputs["u"], np.float32)
    v = np.asarray(inputs["v"], np.float32)
    Wo = np.asarray(inputs["Wo"], np.float32)

    xTb = [np.ascontiguousarray(x[b, :S].T).astype(bf16) for b in range(B_FULL)]
    posTb = [np.ascontiguousarray(pos[b, :S].T).astype(bf16) for b in range(B_FULL)]

    in_maps = []
    for c in range(n_cores):
        b = c // 4
        h0 = 2 * (c % 4)
        sl = slice(h0 * D_HEAD, (h0 + 2) * D_HEAD)
        u_eff = ((u[h0 : h0 + 2].reshape(-1) + bq[sl]) * ISQ).astype(np.float32)
        v_eff = ((v[h0 : h0 + 2].reshape(-1) + bq[sl]) * ISQ).astype(np.float32)
        in_maps.append(
            {
                "xT": xTb[b],
                "posT": posTb[b],
                "Wq": np.ascontiguousarray(Wq[:, sl]).astype(bf16),
                "Wk": np.ascontiguousarray(Wk[:, sl]).astype(bf16),
                "Wv": np.ascontiguousarray(Wv[:, sl]).astype(bf16),
                "Wp": np.ascontiguousarray(Wp[:, sl]).astype(bf16),
                "Wo": np.ascontiguousarray(Wo[sl, :]).astype(bf16),
                "ueff": u_eff.reshape(DH2, 1),
                "veff": v_eff.reshape(DH2, 1),
            }
        )
    return in_maps


def assemble(inputs, results, S=S_FULL):
    bv = np.asarray(inputs["bv"], np.float64)
    Wo = np.asarray(inputs["Wo"], np.float64)
    bo = np.asarray(inputs["bo"], np.float64)
    const = (bv @ Wo + bo).astype(np.float32)
    out = np.zeros((B_FULL, S, D_MODEL), np.float32)
    for c, res in enumerate(results):
        out[c // 4] += np.asarray(res["out_partial"], dtype=np.float32)
    out += const[None, None, :]
    return out


def _run(inputs, trace=False, **kw):
    nc = _get_nc(S_FULL)
    in_maps = make_in_maps(inputs, S_FULL)
    res = run_bass_kernel_spmd(nc, in_maps, list(range(8)), trace=trace, **kw)
    out = assemble(inputs, res.results, S_FULL)
    return out, res


def kernel(**inputs) -> np.ndarray:
    out, _ = _run(inputs, trace=False)
    return out


# revision 7
# speedup vs baseline: 1.1687x; 1.0015x over previous
"""Trainium2 Bass kernel for Transformer-XL style relative-position MHSA.

Problem: nn_MultiHeadSelfAttention_14989435863450
  B=2, S=2048, D=512, H=8, dh=64, fp32 I/O.

Sharding (8 cores): core c -> batch b = c//4, head pair h0 = 2*(c%4).
Each core computes its 2 heads' attention and the partial output
projection; host sums 4 partials per batch and adds (bv @ Wo + bo).

Math folds (exact):
  - bq folds into u,v:  u_eff = (u + bq) / sqrt(D)
  - bk adds a per-query-row constant to scores -> cancels in softmax
  - bv contributes attn-weighted 1 * bv = bv -> host-side constant
  - 1/sqrt(D) folded into q at evacuation time

v2 design (pipelined waves; see kernel_baseline.py for the v1 layout):
  - Scores built TRANSPOSED (sT[k, q]) so attn@v needs no transpose.
  - Rel-shift via DRAM bounce, but PB is SPLIT at the half boundary:
    PBa[h] rows 0..1151 (q-blocks 0-8) serves half 0's shifted reads,
    PBb[h] rows 1024..2047 (blocks 8-15, block 8 written twice) serves
    half 1.  Reads of one half therefore never chain behind writes for
    the other half (whole-tile dep tracking).
  - Ring assignment: sync HWDGE ring carries ONLY the XBAR transposed
    panel reads (concurrent XBAR transposes on both rings race on the
    shared transpose unit -> single ring).  h0's PB writes ride the
    gpsimd SWDGE queue, h1's the scalar HWDGE ring; input loads + out
    writes also scalar.  This kills the v1 serialization where the
    whole main loop queued behind all 32 PB writes on one ring.
  - 4 waves (h0,f0) (h1,f0) (h0,f1) (h1,f1), each: 16 XBAR panel reads
    issued up front, then 16 kt-steps (content matmul -> +pos panel add
    (DVE, every 4th on gpsimd) -> exp (Act) -> attn@v one step late so
    exp latency is off the PE critical path).  Wave k+1's pos blocks /
    projections are interleaved into earlier waves' kt-steps to keep
    the PE continuously busy (pstate ramps to 2.4GHz after ~3us busy;
    idle drops it to 1.2GHz - the v1 killer).
  - Input x/pos loaded as 4 per-chunk SBUF tiles so projections start
    while later chunks stream.
  - out_partial is bf16 (host accumulates in fp32): halves out traffic.
"""

import math
from contextlib import ExitStack

import numpy as np
import ml_dtypes

import concourse.bass as bass
import concourse.bacc as bacc_mod
import concourse.mybir as mybir
import concourse.tile as tile
from concourse.bass import ts, ds
from concourse.bass_utils import run_bass_kernel_spmd

FP32 = mybir.dt.float32
BF16 = mybir.dt.bfloat16

D_MODEL = 512
NUM_HEADS = 8
D_HEAD = 64
DH2 = 2 * D_HEAD
B_FULL = 2
S_FULL = 2048
P = 128
CH = 512
ISQ = 1.0 / math.sqrt(D_MODEL)

Exp = mybir.ActivationFunctionType.Exp
ADD = mybir.AluOpType.add
MULT = mybir.AluOpType.mult


def build_nc(S=S_FULL):
    nc = bacc_mod.Bacc()
    NB = S // P        # 16 q blocks
    NK = S // P        # 16 k tiles
    NCH = S // CH      # 4 chunks
    KD = D_MODEL // P  # 4
    HALF = S // 2      # 1024
    NBH = NB // 2      # 8 q blocks per half

    xT = nc.declare_dram_parameter("xT", [D_MODEL, S], BF16, isOutput=False)
    posT = nc.declare_dram_parameter("posT", [D_MODEL, S], BF16, isOutput=False)
    Wq = nc.declare_dram_parameter("Wq", [D_MODEL, DH2], BF16, isOutput=False)
    Wk = nc.declare_dram_parameter("Wk", [D_MODEL, DH2], BF16, isOutput=False)
    Wv = nc.declare_dram_parameter("Wv", [D_MODEL, DH2], BF16, isOutput=False)
    Wp = nc.declare_dram_parameter("Wp", [D_MODEL, DH2], BF16, isOutput=False)
    Wo = nc.declare_dram_parameter("Wo", [DH2, D_MODEL], BF16, isOutput=False)
    ueff = nc.declare_dram_parameter("ueff", [DH2, 1], FP32, isOutput=False)
    veff = nc.declare_dram_parameter("veff", [DH2, 1], FP32, isOutput=False)
    out_partial = nc.declare_dram_parameter("out_partial", [S, D_MODEL], BF16, isOutput=True)

    with ExitStack() as ctx:
        tc = ctx.enter_context(tile.TileContext(nc))
        consts = ctx.enter_context(tc.tile_pool(name="consts", bufs=1))
        blk = ctx.enter_context(tc.tile_pool(name="blk", bufs=3))
        spool = ctx.enter_context(tc.tile_pool(name="spool", bufs=34))
        dram = ctx.enter_context(tc.tile_pool(name="dram", bufs=1, space="DRAM"))
        # PSUM (8 banks): psAcc 1x[65,2,512] (2) + psC 3x[128,2,512] (6)
        psAcc = ctx.enter_context(tc.tile_pool(name="psAcc", bufs=1, space="PSUM"))
        psC = ctx.enter_context(tc.tile_pool(name="psC", bufs=3, space="PSUM"))

        # ---- loads: all on the scalar HWDGE ring, chunked for early start
        w_sbs = {}

        def load_w(nm, handle):
            w_sb = consts.tile([P, KD, DH2], BF16, name=f"{nm}_sb")
            nc.scalar.dma_start(w_sb[:], handle.rearrange("(o p) m -> p o m", p=P))
            w_sbs[nm] = w_sb

        xc = []
        pc = []
        load_w("Wq", Wq)
        xv = xT.rearrange("(o p) s -> p o s", p=P)
        pv = posT.rearrange("(o p) s -> p o s", p=P)
        for c in range(NCH):
            t = consts.tile([P, KD, CH], BF16, name=f"xc{c}")
            nc.scalar.dma_start(t[:], xv[:, :, ts(c, CH)])
            xc.append(t)
        load_w("Wp", Wp)
        for c in range(NCH):
            t = consts.tile([P, KD, CH], BF16, name=f"pc{c}")
            nc.scalar.dma_start(t[:], pv[:, :, ts(c, CH)])
            pc.append(t)
        load_w("Wk", Wk)
        load_w("Wv", Wv)
        Wo_sb = consts.tile([D_HEAD, 2, D_MODEL], BF16)
        nc.scalar.dma_start(Wo_sb[:], Wo.rearrange("(h d) n -> d h n", h=2))
        ueff_sb = consts.tile([DH2, 1], FP32)
        nc.scalar.dma_start(ueff_sb[:], ueff[:, :])
        veff_sb = consts.tile([DH2, 1], FP32)
        nc.scalar.dma_start(veff_sb[:], veff[:, :])

        qTu = consts.tile([DH2, S], BF16)
        qTv = consts.tile([DH2, S], BF16)
        kT = consts.tile([DH2, S], BF16)
        pT = consts.tile([DH2, S], BF16)
        vv_aug = consts.tile([P, NK, 2, D_HEAD + 1], BF16)
        ones_st = consts.tile([P, NK * 2], FP32)
        nc.vector.memset(ones_st[:], 1.0)
        nc.vector.tensor_copy(
            vv_aug[:, :, :, D_HEAD : D_HEAD + 1],
            ones_st[:].rearrange("p (a b c) -> p a b c", a=NK, b=2),
        )

        # ---- projections (per group g of 2 chunks) ----
        def proj_group(w_sb, src, g, evac):
            pg = psC.tile([P, 2, CH], FP32, tag="ps", name="pg")
            for j in range(2):
                chn = 2 * g + j
                for kt in range(KD):
                    nc.tensor.matmul(
                        pg[:, j, :],
                        lhsT=w_sb[:, kt, :],
                        rhs=src[chn][:, kt, :],
                        start=(kt == 0),
                        stop=(kt == KD - 1),
                    )
            evac(g, pg)

        def evac_q(g, pg):
            sl = ds(g * 2 * CH, 2 * CH)
            pv_ = pg[:].rearrange("p a b -> p (a b)")
            nc.vector.tensor_scalar(qTu[:, sl], pv_, ISQ, ueff_sb[:, 0:1], MULT, ADD)
            nc.vector.tensor_scalar(qTv[:, sl], pv_, ISQ, veff_sb[:, 0:1], MULT, ADD)

        def evac_to(dst):
            def evac(g, pg):
                sl = ds(g * 2 * CH, 2 * CH)
                nc.scalar.copy(dst[:, sl], pg[:].rearrange("p a b -> p (a b)"))
            return evac

        def proj_v(sg):
            pvv = psC.tile([P, 2, CH], FP32, tag="ps", name="pvv")
            for j in range(2):
                st = 2 * sg + j
                for kt in range(KD):
                    nc.tensor.matmul(
                        pvv[:, j, 0:DH2],
                        lhsT=xc[st // 4][:, kt, ts(st % 4, P)],
                        rhs=w_sbs["Wv"][:, kt, :],
                        start=(kt == 0),
                        stop=(kt == KD - 1),
                    )
            for j in range(2):
                st = 2 * sg + j
                src = pvv[:, j, 0:DH2].rearrange("p (h d) -> p h d", h=2)
                nc.vector.tensor_copy(vv_aug[:, st, :, 0:D_HEAD], src)

        # ---- pos score DRAM buffers, split at the half boundary ----
        # PBa[h]: q rows 0..(NBH+1)*P-1  (blocks 0..8)  -> half-0 reads
        # PBb[h]: q rows HALF..S-1       (blocks 8..15) -> half-1 reads
        PBa = [dram.tile([(NBH + 1) * P, S + 1], BF16, name=f"pba{h}") for h in range(2)]
        PBb = [dram.tile([HALF, S + 1], BF16, name=f"pbb{h}") for h in range(2)]

        def pos_block(ib):
            """pos scores for q rows [128*ib, +128), BOTH heads jointly:
            the two heads' matmuls sit at PE array rows 0-63 and 64-127
            (lhsT partition offset), so consecutive pairs overlap.

            h0's evacs + PB write are emitted FIRST so h0's buffers (the
            gate for the next wave's panel reads) complete before h1's
            trailing work.  h0 writes ride the scalar HWDGE ring (fast
            dispatch); h1 rides the gpsimd SWDGE queue (slow dispatch,
            but h1's deadline is one wave later)."""
            pes = [
                blk.tile([P, S + 1], BF16, tag=f"posext{h}", name="pe")
                for h in range(2)
            ]
            for h in range(2):
                nc.vector.memset(pes[h][:, 0:1], 0.0)
            pps_all = []
            for g in range(NCH // 2):
                pps = [
                    psC.tile([P, 2, CH], FP32, tag="ps", name="pp")
                    for _ in range(2)
                ]
                for h in range(2):
                    for j in range(2):
                        chn = 2 * g + j
                        nc.tensor.matmul(
                            pps[h][:, j, :],
                            lhsT=qTv[ds(h * D_HEAD, D_HEAD), ts(ib, P)],
                            rhs=pT[ds(h * D_HEAD, D_HEAD), ts(chn, CH)],
                            start=True,
                            stop=True,
                        )
                pps_all.append(pps)
            for h in range(2):
                for g in range(NCH // 2):
                    dst = pes[h][:, ds(1 + g * 2 * CH, 2 * CH)]
                    src = pps_all[g][h][:].rearrange("p a b -> p (a b)")
                    if (h + g) % 2 == 0:
                        nc.vector.tensor_copy(dst, src)
                    else:
                        nc.scalar.copy(dst, src)
                eng = nc.scalar if h == 0 else nc.gpsimd
                if ib <= NBH:
                    eng.dma_start(PBa[h][ts(ib, P), :], pes[h][:])
                if ib >= NBH:
                    eng.dma_start(PBb[h][ts(ib - NBH, P), :], pes[h][:])

        # unnormalized attn@v results per head (d rows), Z staged separately
        o2u = {}
        zq = {}
        rz = {}
        for h in range(2):
            o2u[h] = blk.tile([D_HEAD, NCH, CH], BF16, tag=f"o2_{h}", bufs=1, name="o2u")
            zq[h] = blk.tile([1, NCH, CH], BF16, tag=f"zq_{h}", bufs=1, name="zq")
            rz[h] = blk.tile([P, NB], FP32, tag=f"rz_{h}", bufs=1, name="rz")
        zd = dram.tile([2, S], BF16, name="zd")

        def issue_read(h, half, kt):
            """prefetch the shifted+transposed pos panel for (h, half, kt)."""
            sp = spool.tile([P, 2, CH], BF16, tag="spos", name="sp")
            if half == 0:
                flat = PBa[h].flatten()
                qview = flat[ds(S, HALF * S)].rearrange("(q k) -> q k", k=S)
            else:
                flat = PBb[h].flatten()
                qview = flat[ds(HALF, HALF * S)].rearrange("(q k) -> q k", k=S)
            nc.sync.dma_start(sp[:].rearrange("p a b -> p (a b)"),
                              qview[:, ts(kt, P)], transpose=True)
            return sp

        def po_step(h, kt, po, et):
            for j in range(2):
                nc.tensor.matmul(
                    po[:, j, :],
                    lhsT=vv_aug[:, kt, h, :],
                    rhs=et[:, j, :],
                    start=(kt == 0),
                    stop=(kt == NK - 1),
                )

        def kt_step(h, half, kt, po, sp, prev, inter):
            """content scores + exp for k-tile kt; attn@v for k-tile kt-1
            (delayed one step so exp(kt-1) is off the PE critical path)."""
            ps = psC.tile([P, 2, CH], FP32, tag="ps", name="ps")
            for j in range(2):
                c = 2 * half + j
                nc.tensor.matmul(
                    ps[:, j, :],
                    lhsT=kT[ds(h * D_HEAD, D_HEAD), ts(kt, P)],
                    rhs=qTu[ds(h * D_HEAD, D_HEAD), ts(c, CH)],
                    start=True,
                    stop=True,
                )
            for fn in inter:
                fn()
            if prev is not None:
                po_step(h, prev[0], po, prev[1])
            sc = blk.tile([P, 2, CH], BF16, tag="sc", name="sc")
            nc.vector.tensor_tensor(sc[:], ps[:], sp[:], ADD)
            et = blk.tile([P, 2, CH], BF16, tag="et", name="et")
            nc.scalar.activation(et[:], sc[:], Exp)
            return (kt, et)

        def wave(h, half, inter_map):
            po = psAcc.tile([D_HEAD + 1, 2, CH], FP32, tag="po", name="po")
            sps = {kt: issue_read(h, half, kt) for kt in range(NK)}
            prev = None
            for kt in range(NK):
                prev = kt_step(h, half, kt, po, sps.pop(kt), prev,
                               inter_map.get(kt, ()))
            po_step(h, prev[0], po, prev[1])
            # evac: d rows -> o2u (DVE), Z row -> zq (Act)
            nc.vector.tensor_copy(o2u[h][:, ts(half, 2), :], po[0:D_HEAD])
            nc.scalar.copy(zq[h][:, ts(half, 2), :], po[D_HEAD : D_HEAD + 1])

        def finish_head(h):
            """Z -> DRAM -> xbar-transposed [128, 16] -> rz = 1/Z."""
            nc.scalar.dma_start(
                zd[h : h + 1, :], zq[h][:].rearrange("p a b -> p (a b)")
            )
            zview = zd.flatten()[ds(h * S, S)].rearrange("(a b) -> a b", b=P)
            rzt = blk.tile([P, NB], BF16, tag=f"rzt_{h}", bufs=1, name="rzt")
            nc.sync.dma_start(rzt[:], zview, transpose=True)
            nc.vector.reciprocal(rz[h][:], rzt[:])

        def pw_block(ib):
            c, j = ib // NCH, ib % NCH
            pw = psC.tile([P, 2, CH], FP32, tag="ps", name="pw")
            for h in range(2):
                nc.tensor.matmul(
                    pw[:, h, :],
                    lhsT=o2u[h][:, c, ts(j, P)],
                    rhs=Wo_sb[:, h, :],
                    start=True,
                    stop=True,
                )
            t1 = blk.tile([P, D_MODEL], FP32, tag="t1", name="t1")
            nc.scalar.mul(t1[:], pw[:, 1, :], rz[1][:, ib : ib + 1])
            fin = blk.tile([P, D_MODEL], BF16, tag="fin", name="fin")
            nc.vector.scalar_tensor_tensor(
                fin[:], pw[:, 0, :], rz[0][:, ib : ib + 1], t1[:], MULT, ADD
            )
            nc.scalar.dma_start(out_partial[ts(ib, P), :], fin[:])

        # ---- prologue: minimal PE work before pos blocks 0-8 so wave 1
        # (gated on PBa writes) starts as early as possible ----
        proj_group(w_sbs["Wq"], xc, 0, evac_q)
        proj_group(w_sbs["Wp"], pc, 0, evac_to(pT))
        proj_group(w_sbs["Wp"], pc, 1, evac_to(pT))
        pos_block(0)
        proj_group(w_sbs["Wq"], xc, 1, evac_q)   # qTv g1 needed by block 8+
        pos_block(1)
        proj_group(w_sbs["Wk"], xc, 0, evac_to(kT))  # kt 0-7 content
        for ib in range(2, NBH + 1):
            pos_block(ib)

        # ---- waves ----
        w1_inter = {
            0: (lambda: proj_v(0), lambda: proj_v(1)),
            2: (lambda: proj_v(2),),
            4: (lambda: proj_v(3),
                lambda: proj_group(w_sbs["Wk"], xc, 1, evac_to(kT))),
            6: (lambda: proj_v(4),),
            8: (lambda: proj_v(5), lambda: pos_block(9)),
            11: (lambda: proj_v(6),),
            12: (lambda: pos_block(10),),
            14: (lambda: proj_v(7),),
        }
        # pos blocks 9-15 spread over waves 1-2 (writes must land before
        # wave 3's reads of PBb[0])
        w2_inter = {
            1: (lambda: pos_block(11),),
            4: (lambda: pos_block(12),),
            7: (lambda: pos_block(13),),
            10: (lambda: pos_block(14),),
            13: (lambda: pos_block(15),),
        }
        w4_inter = {
            1: (lambda: finish_head(0),),
        }
        wave(0, 0, w1_inter)
        wave(1, 0, w2_inter)
        wave(0, 1, {})
        wave(1, 1, w4_inter)

        finish_head(1)
        for ib in range(NB):
            pw_block(ib)

    nc.finalize()
    return nc


# ---------------- host side ----------------

_NC_CACHE = {}


def _get_nc(S=S_FULL):
    if S not in _NC_CACHE:
        _NC_CACHE[S] = build_nc(S)
    return _NC_CACHE[S]


def make_in_maps(inputs, S=S_FULL, n_cores=8):
    bf16 = ml_dtypes.bfloat16
    x = np.asarray(inputs["x"], np.float32)
    pos = np.asarray(inputs["pos_embedding"], np.float32)
    Wq = np.asarray(inputs["Wq"], np.float32)
    bq = np.asarray(inputs["bq"], np.float32)
    Wk = np.asarray(inputs["Wk"], np.float32)
    Wv = np.asarray(inputs["Wv"], np.float32)
    Wp = np.asarray(inputs["Wp"], np.float32)
    u = np.asarray(inputs["u"], np.float32)
    v = np.asarray(inputs["v"], np.float32)
    Wo = np.asarray(inputs["Wo"], np.float32)

    xTb = [np.ascontiguousarray(x[b, :S].T).astype(bf16) for b in range(B_FULL)]
    posTb = [np.ascontiguousarray(pos[b, :S].T).astype(bf16) for b in range(B_FULL)]

    in_maps = []
    for c in range(n_cores):
        b = c // 4
        h0 = 2 * (c % 4)
        sl = slice(h0 * D_HEAD, (h0 + 2) * D_HEAD)
        u_eff = ((u[h0 : h0 + 2].reshape(-1) + bq[sl]) * ISQ).astype(np.float32)
        v_eff = ((v[h0 : h0 + 2].reshape(-1) + bq[sl]) * ISQ).astype(np.float32)
        in_maps.append(
            {
                "xT": xTb[b],
                "posT": posTb[b],
                "Wq": np.ascontiguousarray(Wq[:, sl]).astype(bf16),
                "Wk": np.ascontiguousarray(Wk[:, sl]).astype(bf16),
                "Wv": np.ascontiguousarray(Wv[:, sl]).astype(bf16),
                "Wp": np.ascontiguousarray(Wp[:, sl]).astype(bf16),
                "Wo": np.ascontiguousarray(Wo[sl, :]).astype(bf16),
                "ueff": u_eff.reshape(DH2, 1),
                "veff": v_eff.reshape(DH2, 1),
            }
        )
    return in_maps


def assemble(inputs, results, S=S_FULL):
    bv = np.asarray(inputs["bv"], np.float64)
    Wo = np.asarray(inputs["Wo"], np.float64)
    bo = np.asarray(inputs["bo"], np.float64)
    const = (bv @ Wo + bo).astype(np.float32)
    out = np.zeros((B_FULL, S, D_MODEL), np.float32)
    for c, res in enumerate(results):
        out[c // 4] += np.asarray(res["out_partial"], dtype=np.float32)
    out += const[None, None, :]
    return out


def _run(inputs, trace=False, **kw):
    nc = _get_nc(S_FULL)
    in_maps = make_in_maps(inputs, S_FULL)
    res = run_bass_kernel_spmd(nc, in_maps, list(range(8)), trace=trace, **kw)
    out = assemble(inputs, res.results, S_FULL)
    return out, res


def kernel(**inputs) -> np.ndarray:
    out, _ = _run(inputs, trace=False)
    return out
